# revision 30
# baseline (speedup 1.0000x reference)
"""Trainium2 Bass kernel for AuxiliaryMultiHeadedAttention.

Reference computation (B=4, L=2048, H=256, NH=8, DH=32):
    kb   = split_heads(k_b @ Wb.T + bb)
    corr = (qh @ kh^T + qh @ kb^T) / sqrt(DH) * scale_w[h, q]
    corr = where(mask==0, -1e9, corr);  prob = softmax(corr)
    out  = merge_heads(prob @ vh) @ Ww.T + bw

Kernel strategy (8 NeuronCores):
    Shard (batch, query-half): core c -> batch c//2, queries (c%2)*1024..+1024.
    Each core:
      keffT = (k + k_b @ Wb.T + bb)^T          [dims, keys]  bf16 (dual QK^T folded)
      qsT   = (q * scale_w/sqrt(DH))^T         [dims, queries]  bf16
      S^T   = keffT_h^T @ qsT_h  (bf16, 2 heads row-tiled on PE, own banks)
      P^T   = exp(S^T)  (ACT does ONLY exp; no max-subtract: |logits| < ~40)
      PV with weights [v_h*mask | mask-reps] -> psum [64, 512]:
            rows 0:32 = O^T (unnormalized), rows 32:64 = softmax denominator
      hidT  = O^T * reciprocal(denominator)    (DVE, rsum staged via SBUF)
      out   = hidT^T @ Ww.T + bw               (PE, bf16)
    Host concatenates the 8 [1024, 256] slices.

The main loop is ACT-bound (exp of 16.8M scores per core at 1 elem/lane/cycle
@1.2GHz ~= 142us). All prep PSUM traffic goes through the main loop's own
tile pools (tag "st") so the scheduler can slide prep under the exp stream:
dims-half 0 of keffT/qsT is built first (feeds head-groups g0/g1), dims-half
1 is built inside g0's PE slack. DMA is issued quad-of-chunks first so the
first S matmul only waits on ~2MiB.
"""

import sys

if "/opt/trn_rl_repo" not in sys.path:
    sys.path.insert(0, "/opt/trn_rl_repo")

import math

import numpy as np

B, L, H, NH, DH = 4, 2048, 256, 8, 32
LQ = 1024  # queries per core
NCORES = 8
ISQ = 1.0 / math.sqrt(DH)


def _build():
    import concourse.bass as bass  # noqa: F401
    import concourse.mybir as mybir
    import concourse.tile as tile
    from concourse import bacc

    f32 = mybir.dt.float32
    i32 = mybir.dt.int32
    bf16 = mybir.dt.bfloat16
    Exp = mybir.ActivationFunctionType.Exp
    Alu = mybir.AluOpType

    nc = bacc.Bacc("TRN2", target_bir_lowering=False, debug=False, num_devices=NCORES)

    q_d = nc.dram_tensor("q_s", [LQ, H], f32, kind="ExternalInput")
    k_d = nc.dram_tensor("k_s", [L, H], f32, kind="ExternalInput")
    v_d = nc.dram_tensor("v_s", [L, H], f32, kind="ExternalInput")
    kb_d = nc.dram_tensor("kb_s", [L, H], f32, kind="ExternalInput")
    mask_d = nc.dram_tensor("mask_s", [L], i32, kind="ExternalInput")
    sw_d = nc.dram_tensor("sw_s", [NH, LQ], f32, kind="ExternalInput")
    Wb_d = nc.dram_tensor("Wb", [H, H], f32, kind="ExternalInput")
    bb_d = nc.dram_tensor("bb", [H], f32, kind="ExternalInput")
    Ww_d = nc.dram_tensor("Ww", [H, H], f32, kind="ExternalInput")
    bw_d = nc.dram_tensor("bw", [H], f32, kind="ExternalInput")
    id_d = nc.dram_tensor("ident", [128, 128], f32, kind="ExternalInput")
    out_d = nc.dram_tensor("out", [LQ, H], f32, kind="ExternalOutput")

    with tile.TileContext(nc) as tc:
        with (
            tc.tile_pool(name="persist", bufs=1) as pp,
            tc.tile_pool(name="pt", bufs=6) as ptp,
            tc.tile_pool(name="small", bufs=2) as smp,
            tc.tile_pool(name="pst", bufs=3, space="PSUM") as pst,
            tc.tile_pool(name="ppv", bufs=2, space="PSUM") as ppv,
        ):
            # ---------------- persistent SBUF tensors ----------------
            ident = pp.tile([128, 128], f32, tag="ident")
            nc.sync.dma_start(out=ident, in_=id_d[:, :])
            keffT = [pp.tile([128, L], bf16, tag=f"keffT{g}", name=f"keffT{g}")
                     for g in range(2)]
            qsT = [pp.tile([128, LQ], bf16, tag=f"qsT{g}", name=f"qsT{g}")
                   for g in range(2)]
            # kbTall: col = lc*256 + ec*128 + j  (quad-contiguous pcopies)
            kbTall = pp.tile([128, 2 * L], bf16, tag="kbTall")
            # per (key-chunk, head): [v_hi*mask | mask] -> [128, 64] bf16
            vmm = pp.tile([128, 16 * NH * 64], bf16, tag="vmm")
            hidT = [pp.tile([128, LQ], bf16, tag=f"hidT{g}", name=f"hidT{g}")
                    for g in range(2)]
            # WbTall: col = (ec*2+dc)*128 ; wwT: col = g*256 + er*128
            WbTall = pp.tile([128, 512], bf16, tag="WbTall")
            wwT = pp.tile([128, 512], bf16, tag="wwT")
            ones1 = pp.tile([1, 128], bf16, tag="ones1")
            nc.vector.memset(ones1, 1.0)
            # dummy exp: pulls ACT_TABLE_LOAD (~2.7us) off the critical path
            warm = pp.tile([1, 128], f32, tag="warm")
            nc.scalar.activation(warm, ones1, Exp)
            onesl = pp.tile([1, L], bf16, tag="onesl")
            nc.vector.memset(onesl, 1.0)
            bwb = pp.tile([1, H], bf16, tag="bwb")
            bbb = pp.tile([1, H], bf16, tag="bbb")
            sc8 = pp.tile([128, 64], f32, tag="sc8")
            maskf = pp.tile([128, 16], f32, tag="maskf")
            outsb = pp.tile([128, 8 * H], f32, tag="outsb")

            # ---------------- staging loads (critical path first) ----
            m16 = pp.tile([16, 128], i32, tag="m16")
            nc.sync.dma_start(out=m16,
                              in_=mask_d.rearrange("(c p) -> c p", p=128))
            swt = pp.tile([NH, LQ], f32, tag="swt")
            nc.sync.dma_start(out=swt, in_=sw_d[:, :])
            wbraw = pp.tile([128, 2 * H], f32, tag="wbraw")
            nc.sync.dma_start(out=wbraw.rearrange("p (c e) -> p c e", c=2),
                              in_=Wb_d.rearrange("(c p) e -> p c e", p=128))
            bbt = pp.tile([1, H], f32, tag="bbt")
            nc.sync.dma_start(out=bbt, in_=bb_d[None, :])
            kbraw = pp.tile([128, 16 * H], f32, tag="kbraw")
            kraw = pp.tile([128, 16 * H], f32, tag="kraw")
            vraw = pp.tile([128, 16 * H], f32, tag="vraw")
            qraw = pp.tile([128, 8 * H], f32, tag="qraw")

            def quad_dma(tile_, dram, c4):
                tv = tile_.rearrange("p (c e) -> p c e", c=16)
                dv = dram.rearrange("(c p) e -> p c e", p=128)
                nc.sync.dma_start(out=tv[:, c4 * 4:(c4 + 1) * 4, :],
                                  in_=dv[:, c4 * 4:(c4 + 1) * 4, :])

            def q_dma(half):
                qv = qraw.rearrange("p (c e) -> p c e", c=8)
                dv = q_d.rearrange("(c p) e -> p c e", p=128)
                nc.sync.dma_start(out=qv[:, half * 4:(half + 1) * 4, :],
                                  in_=dv[:, half * 4:(half + 1) * 4, :])

            # arrival order tuned to the consumption order of the prep chain
            quad_dma(kbraw, kb_d, 0)
            quad_dma(kraw, k_d, 0)
            q_dma(0)
            q_dma(1)
            quad_dma(kbraw, kb_d, 1)
            quad_dma(kraw, k_d, 1)
            quad_dma(vraw, v_d, 0)
            quad_dma(kbraw, kb_d, 2)
            quad_dma(kraw, k_d, 2)
            quad_dma(vraw, v_d, 1)
            quad_dma(kbraw, kb_d, 3)
            quad_dma(kraw, k_d, 3)
            quad_dma(vraw, v_d, 2)
            quad_dma(vraw, v_d, 3)
            wwraw = pp.tile([128, 2 * H], f32, tag="wwraw")
            nc.sync.dma_start(out=wwraw.rearrange("p (c e) -> p c e", c=2),
                              in_=Ww_d.rearrange("(c p) e -> p c e", p=128))
            bwt = pp.tile([1, H], f32, tag="bwt")
            nc.sync.dma_start(out=bwt, in_=bw_d[None, :])

            m16f = pp.tile([16, 128], f32, tag="m16f")
            nc.vector.tensor_copy(m16f, m16)
            nc.vector.tensor_copy(bbb, bbt)
            nc.vector.tensor_copy(bwb, bwt)

            # ---------------- prep, all through the "st" psum tag ------
            # PSUM flag discipline: start=True clears has_written for the
            # whole 512-col bank (per written partition), so only the FIRST
            # write per bank starts; later writes to unwritten cols overwrite.
            def tpose(out, in_, identity, start, stop):
                nc.tensor.matmul(out, lhsT=in_, rhs=identity,
                                 is_transpose=True, start=start, stop=stop)

            # tile A: WbTall quarters (bank 0) + sc8 + maskT (bank 1)
            ta = pst.tile([128, 1024], f32, tag="st", name="ta")
            for dc in range(2):
                for ec in range(2):
                    qtr = ec * 2 + dc
                    tpose(ta[:, qtr * 128:(qtr + 1) * 128],
                          wbraw[:, dc * H + ec * 128: dc * H + (ec + 1) * 128],
                          ident, start=(qtr == 0), stop=(qtr == 3))
            for mq in range(8):
                tpose(ta[:, 512 + mq * 8:512 + (mq + 1) * 8],
                      swt[:, mq * 128:(mq + 1) * 128],
                      ident[0:NH, 0:NH], start=(mq == 0), stop=False)
            tpose(ta[:, 576:592], m16f, ident[0:16, 0:16],
                  start=False, stop=True)
            # ACT owns all pre-loop PSUM evacuations (it is idle until the
            # first exp); DVE keeps qscale + small casts
            nc.scalar.copy(WbTall, ta[:, 0:512])
            nc.scalar.copy(sc8, ta[:, 512:576])
            nc.scalar.copy(maskf, ta[:, 576:592])

            # q scale by scale_w/sqrt(DH) in place (DVE)
            for mq in range(8):
                qv = qraw[:, mq * H:(mq + 1) * H].rearrange(
                    "p (h j) -> p h j", h=NH)
                nc.vector.scalar_tensor_tensor(
                    out=qv, in0=qv, scalar=ISQ,
                    in1=sc8[:, mq * 8:(mq + 1) * 8][:, :, None].broadcast_to(
                        [128, 8, 32]),
                    op0=Alu.mult, op1=Alu.mult)

            # kbT quads + vmm quads
            vmm4 = vmm.rearrange("p (c h w) -> p c h w", c=16, h=NH)
            vraw3 = vraw.rearrange("p (c e) -> p c e", c=16)

            def tk_quad(qd, cp=None):
                tk = pst.tile([128, 1024], f32, tag="st", name=f"tk{qd}")
                for j in range(4):
                    lc = qd * 4 + j
                    for ec in range(2):
                        pos = j * 2 + ec  # 0..7 across the 2 banks
                        tpose(tk[:, j * 256 + ec * 128:j * 256 + (ec + 1) * 128],
                              kbraw[:, lc * H + ec * 128: lc * H + (ec + 1) * 128],
                              ident, start=(pos % 4 == 0), stop=(pos % 4 == 3))
                (cp or nc.vector.tensor_copy)(
                    kbTall[:, qd * 1024:(qd + 1) * 1024], tk)

            def vmm_quad(qd):
                # SBUF->SBUF only, so GpSimd can own it (keeps DVE free for
                # the PSUM evacuations)
                for j in range(4):
                    lc = qd * 4 + j
                    vsl = vraw3[:, lc, :].rearrange("p (h j) -> p h j", h=NH)
                    mb = maskf[:, lc:lc + 1][:, :, None]
                    nc.gpsimd.tensor_tensor(
                        out=vmm4[:, lc, :, 0:32], in0=vsl,
                        in1=mb.broadcast_to([128, NH, 32]), op=Alu.mult)
                    nc.gpsimd.tensor_copy(
                        vmm4[:, lc, :, 32:64],
                        mb.broadcast_to([128, NH, 32]))

            kbv = kbTall.rearrange("p (q l e j) -> p q l e j", q=4, l=4, e=2)

            kf_tiles = {}

            def keff_half_a(dc, hf):
                t = pst.tile([128, 1024], f32, tag="st", name=f"kf{dc}{hf}")
                kf_tiles[(dc, hf)] = t
                for j in range(8):
                    lc = hf * 8 + j
                    nc.tensor.matmul(
                        t[:, j * 128:(j + 1) * 128],
                        lhsT=kraw[:, lc * H + dc * 128: lc * H + (dc + 1) * 128],
                        rhs=ident,
                        is_transpose=True,
                        start=(j % 4 == 0), stop=False)

            def keff_half_b(dc, hf, cp=None):
                t = kf_tiles.pop((dc, hf))
                for ec in range(2):
                    for qh in range(2):
                        qd = hf * 2 + qh
                        nc.tensor.matmul(
                            t[:, qh * 512:(qh + 1) * 512],
                            lhsT=WbTall[:, (ec * 2 + dc) * 128:
                                        (ec * 2 + dc + 1) * 128],
                            rhs=kbv[:, qd, :, ec, :],
                            start=False, stop=False)
                for qh in range(2):
                    nc.tensor.matmul(
                        t[:, qh * 512:(qh + 1) * 512],
                        lhsT=bbb[0:1, dc * 128:(dc + 1) * 128],
                        rhs=onesl[0:1, qh * 512:(qh + 1) * 512],
                        start=False, stop=True)
                (cp or nc.vector.tensor_copy)(
                    keffT[dc][:, hf * 1024:(hf + 1) * 1024], t)

            def keff_half(dc, hf, cp=None):
                keff_half_a(dc, hf)
                keff_half_b(dc, hf, cp)

            def qsT_half(dc, cp=None):
                t = pst.tile([128, 1024], f32, tag="st", name=f"qt{dc}")
                for mq in range(8):
                    tpose(t[:, mq * 128:(mq + 1) * 128],
                          qraw[:, mq * H + dc * 128: mq * H + (dc + 1) * 128],
                          ident, start=(mq % 4 == 0), stop=(mq % 4 == 3))
                (cp or nc.vector.tensor_copy)(qsT[dc], t)

            def ww_prep():
                t = pst.tile([128, 1024], f32, tag="st", name="tw")
                for er in range(2):
                    for g in range(2):
                        qtr = g * 2 + er
                        tpose(t[:, g * 256 + er * 128: g * 256 + (er + 1) * 128],
                              wwraw[:, er * H + g * 128: er * H + (g + 1) * 128],
                              ident, start=(qtr == 0), stop=(qtr == 3))
                nc.vector.tensor_copy(wwT, t[:, 0:512])

            # minimal pre-loop prep: only what g0's first groups consume
            tk_quad(0, cp=nc.scalar.copy)
            tk_quad(1, cp=nc.scalar.copy)
            keff_half(0, 0, cp=nc.scalar.copy)
            qsT_half(0, cp=nc.scalar.copy)
            vmm_quad(0)

            # the rest slides into the main loop's PE/DVE/GpSimd slack, one
            # sub-us step per (g, kc) so the exp stream never stalls long
            pe_steps = {
                (0, 0): lambda: tk_quad(2),
                (0, 3): lambda: tk_quad(3),
                (0, 5): lambda: keff_half_a(0, 1),
                (0, 6): lambda: keff_half_b(0, 1),
                (0, 8): lambda: keff_half_a(1, 0),
                (0, 9): lambda: keff_half_b(1, 0),
                (0, 11): lambda: keff_half_a(1, 1),
                (0, 12): lambda: keff_half_b(1, 1),
                (0, 14): lambda: qsT_half(1),
                (1, 1): ww_prep,
            }
            dve_steps = {
                (0, 1): lambda: vmm_quad(1),
                (0, 5): lambda: vmm_quad(2),
                (0, 9): lambda: vmm_quad(3),
            }

            # ---------------- main attention loop ----------------
            # group g: heads (2g, 2g+1); chunk ch = g//2; rows (g%2)*64 + 32t
            def norm(g, qb):
                ch = g // 2
                for t in range(2):
                    ro = (g % 2) * 64 + t * 32
                    rsum = smp.tile([32, 512], f32, tag="rsum", name="rsum")
                    den = pvs[g][qb][64 * t + 32:64 * t + 64, :]
                    if g == 3 and t == 0:
                        # ACT is past its last exp here; run the two t-chains
                        # on separate engines
                        nc.scalar.copy(rsum, den)
                    else:
                        nc.vector.tensor_copy(rsum, den)
                    rcp = smp.tile([32, 512], f32, tag="rcp", name="rcp")
                    nc.vector.reciprocal_approx_fast(rcp, rsum)
                    nc.vector.tensor_tensor(
                        out=hidT[ch][ro:ro + 32, qb * 512:(qb + 1) * 512],
                        in0=pvs[g][qb][64 * t:64 * t + 32, :],
                        in1=rcp, op=Alu.mult)

            def out_pair(pair):
                # two mq per psum tile; ACT/DVE alternate the evacuations
                po = ppv.tile([128, 512], f32, tag="pv", name=f"po{pair}")
                for i in range(2):
                    mq = pair * 2 + i
                    for g in range(2):
                        nc.tensor.matmul(
                            po[:, i * 256:(i + 1) * 256],
                            lhsT=hidT[g][:, mq * 128:(mq + 1) * 128],
                            rhs=wwT[:, g * 256:(g + 1) * 256],
                            start=(g == 0 and i == 0), stop=False)
                    nc.tensor.matmul(
                        po[:, i * 256:(i + 1) * 256],
                        lhsT=ones1, rhs=bwb, start=False, stop=(i == 1))
                # ACT is idle after its last exp; DVE is busy with norm(3)
                nc.scalar.copy(outsb[:, pair * 512:(pair + 1) * 512], po)
                nc.sync.dma_start(
                    out=out_d.rearrange("(c p) e -> p c e",
                                        p=128)[:, pair * 2:(pair + 1) * 2, :],
                    in_=outsb.rearrange("p (c e) -> p c e",
                                        c=8)[:, pair * 2:(pair + 1) * 2, :])

            def emit_pv(ki):
                g, kc = divmod(ki, 16)
                for t in range(2):
                    h = 2 * g + t
                    for qb in range(2):
                        nc.tensor.matmul(
                            pvs[g][qb][64 * t:64 * t + 64, :],
                            lhsT=vmm[:, (kc * NH + h) * 64:
                                     (kc * NH + h) * 64 + 64],
                            rhs=ptss[ki % 4][qb][:, t * 512:(t + 1) * 512],
                            tile_position=(0, 64 * t),
                            start=(kc == 0), stop=(kc == 15))

            # software-pipelined emission: unit ki emits S(ki), exp(ki),
            # then PV(ki-1), so next-unit S matmuls are never queued behind
            # a PV that waits on the current exp (PE executes in order).
            pvs = {}
            ptss = {}
            for ki in range(64):
                g, kc = divmod(ki, 16)
                ch = g // 2
                if kc == 0:
                    pvs[g] = [ppv.tile([128, 512], f32, tag="pv",
                                       name=f"pv{g}_{qb}") for qb in range(2)]
                step = pe_steps.pop((g, kc), None)
                if step is not None:
                    step()
                sts = [pst.tile([128, 1024], f32, tag="st",
                                name=f"st{qb}") for qb in range(2)]
                for qb in range(2):
                    for t in range(2):
                        ro = (g % 2) * 64 + t * 32
                        nc.tensor.matmul(
                            sts[qb][:, t * 512:(t + 1) * 512],
                            lhsT=keffT[ch][ro:ro + 32,
                                           kc * 128:(kc + 1) * 128],
                            rhs=qsT[ch][ro:ro + 32,
                                        qb * 512:(qb + 1) * 512],
                            tile_position=(ro, 0),
                            start=True, stop=True)
                pts = []
                for qb in range(2):
                    pt = ptp.tile([128, 1024], bf16, tag="pt",
                                  name=f"pt{qb}")
                    nc.scalar.activation(pt, sts[qb], Exp)
                    pts.append(pt)
                ptss[ki % 4] = pts
                step = dve_steps.pop((g, kc), None)
                if step is not None:
                    step()
                if ki > 0:
                    emit_pv(ki - 1)
                if ki % 16 == 0 and ki > 0:
                    norm(g - 1, 0)
                    norm(g - 1, 1)

            # ---------------- tail: last PV + g3 normalize + output ----
            emit_pv(63)
            norm(3, 0)
            out_pair(0)
            norm(3, 1)
            out_pair(1)
            out_pair(2)
            out_pair(3)

    nc.compile()
    return nc


def _make_in_maps(inputs):
    q = np.ascontiguousarray(np.asarray(inputs["q"], dtype=np.float32))
    k = np.ascontiguousarray(np.asarray(inputs["k"], dtype=np.float32))
    v = np.ascontiguousarray(np.asarray(inputs["v"], dtype=np.float32))
    k_b = np.ascontiguousarray(np.asarray(inputs["k_b"], dtype=np.float32))
    mask = np.ascontiguousarray(np.asarray(inputs["mask"], dtype=np.int32))
    sw = np.ascontiguousarray(np.asarray(inputs["scale_w"], dtype=np.float32))
    Wb = np.ascontiguousarray(np.asarray(inputs["Wb"], dtype=np.float32))
    bb = np.ascontiguousarray(np.asarray(inputs["bb"], dtype=np.float32))
    Ww = np.ascontiguousarray(np.asarray(inputs["Ww"], dtype=np.float32))
    bw = np.ascontiguousarray(np.asarray(inputs["bw"], dtype=np.float32))
    ident = np.eye(128, dtype=np.float32)
    in_maps = []
    for c in range(NCORES):
        b, qs = c // 2, c % 2
        in_maps.append({
            "q_s": q[b, qs * LQ:(qs + 1) * LQ, :],
            "k_s": k[b],
            "v_s": v[b],
            "kb_s": k_b[b],
            "mask_s": mask[b],
            "sw_s": np.ascontiguousarray(sw[:, qs * LQ:(qs + 1) * LQ]),
            "Wb": Wb, "bb": bb, "Ww": Ww, "bw": bw,
            "ident": ident,
        })
    return in_maps


def run_sharded(inputs, trace=False, tmpdir=None):
    from concourse import bass_utils
    from concourse.bass_utils import run_bass_kernel_spmd

    if trace:
        _install_ntff_hook()
        bass_utils.upload_artifacts = lambda d: d
    nc = _build()
    in_maps = _make_in_maps(inputs)
    res = run_bass_kernel_spmd(nc, in_maps, list(range(NCORES)),
                               trace=trace, tmpdir=tmpdir)
    out = np.empty((B, L, H), dtype=np.float32)
    for c in range(NCORES):
        b, qs = c // 2, c % 2
        out[b, qs * LQ:(qs + 1) * LQ, :] = res.results[c]["out"]
    return out, res


def kernel(**inputs):
    out, _ = run_sharded(inputs, trace=False)
    return out


def _install_ntff_hook():
    """Provide antenv.axon_hooks (absent in this image) so trace=True works."""
    import contextlib
    import ctypes
    import types

    import antenv

    if hasattr(antenv, "axon_hooks"):
        return
    mod = types.ModuleType("antenv.axon_hooks")
    _hook = [None]
    mod.set_axon_ntff_profile_hook = lambda h: _hook.__setitem__(0, h)
    mod.get_axon_ntff_profile_hook = lambda: _hook[0]
    antenv.axon_hooks = mod
    sys.modules["antenv.axon_hooks"] = mod

    lib = ctypes.CDLL("/opt/axon/libaxon_pjrt.so")
    if not hasattr(lib, "axon_start_nrt_profile"):
        return
    lib.axon_start_nrt_profile.argtypes = [ctypes.POINTER(ctypes.c_int64),
                                           ctypes.c_size_t]
    lib.axon_start_nrt_profile.restype = ctypes.c_int64
    lib.axon_stop_nrt_profile.argtypes = [ctypes.c_char_p]
    lib.axon_stop_nrt_profile.restype = ctypes.c_int64

    @contextlib.contextmanager
    def _profile(output_dir, device_ids):
        import jax

        jax.devices()
        if device_ids:
            ids = (ctypes.c_int64 * len(device_ids))(*device_ids)
            rc = lib.axon_start_nrt_profile(ids, len(device_ids))
        else:
            rc = lib.axon_start_nrt_profile(None, 0)
        if rc != 0:
            raise RuntimeError(f"axon_start_nrt_profile rc={rc}")
        try:
            yield
        finally:
            n = lib.axon_stop_nrt_profile(str(output_dir).encode())
            print(f"profile: {n} file(s) written to {output_dir}",
                  file=sys.stderr)

    mod.set_axon_ntff_profile_hook(_profile)
